# revision 4
# baseline (speedup 1.0000x reference)
"""Trainium2 Bass kernel for nn_PerceiverAttn (rotary partial-query attention).

Reference computes, for x [4,4096,1024], Wq/Wk/Wv [1024,1024], q_len=512:
  k = rot(x @ Wk.T), v = x @ Wv.T over all 4096 positions,
  q = rot(x[:, -512:] @ Wq.T) over the last 512 positions,
  out = softmax(q k^T / 8 + banded-causal mask) v       -> [4, 512, 1024]

Sharding: 8 cores = 4 batches x 2 head-groups (8 heads each). Each core gets
x[b] and its 512-row slice of each weight, computes out[b, :, 512g:512g+512].

Device pipeline (all matmuls bf16, fp32 accumulation / softmax):
  x[b] --SWDGE cast--> bf16 DRAM --DMA-xbar-transpose--> x^T in SBUF
  K/Q-proj emitted transposed (d on partitions) so rotary pair-swap is a
  PE permutation-matmul + 2 DVE muls + add; V-proj emitted natural with a
  baked ones-column so P^T V matmuls also produce the softmax denominator.
  scores^T chunks -> ACT exp -> bf16 P^T -> PV accumulation -> PE transpose,
  reciprocal-scale, one DMA store.
"""
import sys
for _p in ("/opt/trn_rl_repo", "/root/.axon_site", "/root/.axon_site/_ro/pypackages"):
    if _p not in sys.path:
        sys.path.append(_p)

import numpy as np
import ml_dtypes
from contextlib import ExitStack

import concourse.bacc as bacc
import concourse.mybir as mybir
import concourse.tile as tile
from concourse.bass_utils import run_bass_kernel_spmd

F32 = mybir.dt.float32
BF16 = mybir.dt.bfloat16
EXP = mybir.ActivationFunctionType.Exp

S, E, QL = 4096, 1024, 512
NH_LOC, DH = 8, 64          # heads per core, head dim
DALL = NH_LOC * DH          # 512 output dims per core
NC_CH = 8                   # 128-wide chunks of the contraction dim E
NKS = 8                     # 512-wide key slices
NKT = 32                    # 128-wide key tiles
NM = 4                      # 128-wide chunks of DALL (2 heads each)


def _build():
    nc = bacc.Bacc("TRN2", target_bir_lowering=False, debug=False)

    xb = nc.dram_tensor("xb", [S, E], F32, kind="ExternalInput")
    wk = nc.dram_tensor("wk", [DALL, E], F32, kind="ExternalInput")
    wv = nc.dram_tensor("wv", [DALL, E], F32, kind="ExternalInput")
    wq = nc.dram_tensor("wq", [DALL, E], F32, kind="ExternalInput")
    cosk = nc.dram_tensor("cosk", [128, S], BF16, kind="ExternalInput")
    sink = nc.dram_tensor("sink", [128, S], BF16, kind="ExternalInput")
    cosq = nc.dram_tensor("cosq", [128, QL], BF16, kind="ExternalInput")
    sinq = nc.dram_tensor("sinq", [128, QL], BF16, kind="ExternalInput")
    maskm = nc.dram_tensor("maskm", [128, 4, QL], BF16, kind="ExternalInput")
    pswap = nc.dram_tensor("pswap", [128, 128], F32, kind="ExternalInput")
    ident = nc.dram_tensor("ident", [128, 128], F32, kind="ExternalInput")
    out = nc.dram_tensor("out", [QL, DALL], F32, kind="ExternalOutput")

    with tile.TileContext(nc) as tc, ExitStack() as ctx:
        ep = ctx.enter_context
        dram = ep(tc.tile_pool(name="dram", bufs=1, space="DRAM"))
        cpool = ep(tc.tile_pool(name="const", bufs=1))
        kpool = ep(tc.tile_pool(name="kT", bufs=4))
        qpool = ep(tc.tile_pool(name="qT", bufs=1))
        vpool = ep(tc.tile_pool(name="v", bufs=1))
        pspool = ep(tc.tile_pool(name="ps", bufs=8, space="PSUM"))
        # projection-phase pools, released (LIFO) before attention-phase pools
        proj_ctx = ExitStack()
        epp = proj_ctx.enter_context
        wpool = epp(tc.tile_pool(name="wT", bufs=3))
        xpool = epp(tc.tile_pool(name="xT", bufs=8))
        ktfp = epp(tc.tile_pool(name="ktf", bufs=2))
        tmpp = epp(tc.tile_pool(name="rtmp", bufs=2))

        def ps_tile(shape):
            return pspool.tile(shape, F32, tag="ps", name="ps")

        # ---- constants into SBUF (ACT HWDGE ring, parallel to SP ring) ----
        cosk_sb = cpool.tile([128, S], BF16, tag="cosk")
        sink_sb = cpool.tile([128, S], BF16, tag="sink")
        cosq_sb = cpool.tile([128, QL], BF16, tag="cosq")
        sinq_sb = cpool.tile([128, QL], BF16, tag="sinq")
        mask_sb = cpool.tile([128, 4, QL], BF16, tag="mask")
        pswap_sb = cpool.tile([128, 128], F32, tag="pswap")
        ident_sb = cpool.tile([128, 128], F32, tag="ident")
        biasz = cpool.tile([128, 1], F32, tag="biasz")
        for dst, src in [(pswap_sb, pswap), (ident_sb, ident), (cosq_sb, cosq),
                         (sinq_sb, sinq), (mask_sb, maskm), (cosk_sb, cosk),
                         (sink_sb, sink)]:
            nc.scalar.dma_start(out=dst[:], in_=src[:])
        nc.vector.memset(biasz[:], 0.0)

        # v natural layout with a ones column per head: [keys, kt, head, 64+1]
        v_sb = vpool.tile([128, NKT, NH_LOC, DH + 1], BF16, tag="v")
        nc.vector.memset(v_sb[:], 1.0)

        # ---- cast DMAs (SWDGE ring) ----
        wbf = []
        for w_in in (wk, wv, wq):
            t = dram.tile([DALL, E], BF16, name="wbf")
            nc.gpsimd.dma_start(out=t[:], in_=w_in[:])
            wbf.append(t)
        xbf = []
        for cb in range(4):
            t = dram.tile([S, 256], BF16, name="xbf")
            nc.gpsimd.dma_start(out=t[:], in_=xb[:, cb * 256:(cb + 1) * 256])
            xbf.append(t)

        # ---- transposes (SP HWDGE ring), ordered so K-proj can start early --
        wkT = wpool.tile([128, NC_CH, DALL], BF16, tag="w")
        wvT = wpool.tile([128, NC_CH, DALL], BF16, tag="w")
        wqT = wpool.tile([128, NC_CH, DALL], BF16, tag="w")
        for c in range(NC_CH):
            nc.sync.dma_start_transpose(
                out=wkT[:, c, :], in_=wbf[0][:, c * 128:(c + 1) * 128])
        xT = []
        for c in range(NC_CH):
            t = xpool.tile([128, S], BF16, tag="xT", name="xT")
            nc.sync.dma_start_transpose(
                out=t[:], in_=xbf[c // 2][:, (c % 2) * 128:(c % 2 + 1) * 128])
            xT.append(t)
            nc.sync.dma_start_transpose(
                out=wvT[:, c, :], in_=wbf[1][:, c * 128:(c + 1) * 128])
            nc.sync.dma_start_transpose(
                out=wqT[:, c, :], in_=wbf[2][:, c * 128:(c + 1) * 128])

        # ---- rotary helper: dst(bf16) = psum_src*cos + (pswap @ psum_src)*sin
        def rotary(dst_ap, src_psum, cos_ap, sin_ap):
            ktf = ktfp.tile([128, 512], F32, tag="ktf", name="ktf")
            nc.scalar.copy(ktf[:], src_psum[:])
            up = ps_tile([128, 512])
            nc.tensor.matmul(up[:], lhsT=pswap_sb[:], rhs=ktf[:],
                             start=True, stop=True)
            m1 = tmpp.tile([128, 512], F32, tag="rtmp", name="m1")
            m2 = tmpp.tile([128, 512], F32, tag="rtmp", name="m2")
            nc.vector.tensor_mul(m1[:], up[:], sin_ap)
            nc.vector.tensor_mul(m2[:], ktf[:], cos_ap)
            nc.vector.tensor_add(dst_ap, m1[:], m2[:])

        # ---- K projection (transposed out) + rotary, per 128-row chunk m ----
        kT = [kpool.tile([128, S], BF16, tag="kT", name="kT") for _ in range(NM)]
        for m in range(NM):
            for half in range(2):
                pk = [ps_tile([128, 512]) for _ in range(4)]
                for c in range(NC_CH):
                    for j in range(4):
                        ks = half * 4 + j
                        nc.tensor.matmul(
                            pk[j][:],
                            lhsT=wkT[:, c, m * 128:(m + 1) * 128],
                            rhs=xT[c][:, ks * 512:(ks + 1) * 512],
                            start=(c == 0), stop=(c == NC_CH - 1))
                for j in range(4):
                    ks = half * 4 + j
                    rotary(kT[m][:, ks * 512:(ks + 1) * 512], pk[j],
                           cosk_sb[:, ks * 512:(ks + 1) * 512],
                           sink_sb[:, ks * 512:(ks + 1) * 512])

        # ---- Q projection (transposed out, last 512 positions) + rotary ----
        qT = qpool.tile([128, NM, QL], BF16, tag="qT")
        for m in range(NM):
            pq = ps_tile([128, 512])
            for c in range(NC_CH):
                nc.tensor.matmul(pq[:],
                                 lhsT=wqT[:, c, m * 128:(m + 1) * 128],
                                 rhs=xT[c][:, S - QL:],
                                 start=(c == 0), stop=(c == NC_CH - 1))
            rotary(qT[:, m, :], pq, cosq_sb[:], sinq_sb[:])

        # ---- V projection (natural layout) ----
        for kt in range(NKT):
            pv = ps_tile([128, 512])
            for c in range(NC_CH):
                nc.tensor.matmul(pv[:],
                                 lhsT=xT[c][:, kt * 128:(kt + 1) * 128],
                                 rhs=wvT[:, c, :],
                                 start=(c == 0), stop=(c == NC_CH - 1))
            nc.scalar.copy(v_sb[:, kt, :, 0:DH],
                           pv[:].rearrange("p (h d) -> p h d", d=DH))

        # release projection-phase SBUF (xT, wT, rotary temps)
        proj_ctx.close()
        ptp = ep(tc.tile_pool(name="pt", bufs=4))
        opool = ep(tc.tile_pool(name="oT", bufs=2))
        finp = ep(tc.tile_pool(name="ofin", bufs=1))
        rcpool = ep(tc.tile_pool(name="rcp", bufs=4))

        # ---- attention per head: scores^T -> exp -> mask -> P^T V ----
        ofin = finp.tile([128, 4, DALL], F32, tag="ofin")
        for h in range(NH_LOC):
            m, po = h // 2, 64 * (h % 2)
            pav = ps_tile([65, 512])
            for kt in range(NKT):
                sp = ps_tile([128, 512])
                nc.tensor.matmul(sp[:],
                                 lhsT=kT[m][po:po + 64, kt * 128:(kt + 1) * 128],
                                 rhs=qT[po:po + 64, m, :],
                                 start=True, stop=True)
                pt = ptp.tile([128, 512], BF16, tag="pt", name="pt")
                nc.scalar.activation(pt[:], sp[:], EXP, bias=biasz[:], scale=1.0)
                if kt >= 28:
                    nc.vector.tensor_mul(pt[:], pt[:], mask_sb[:, kt - 28, :])
                nc.tensor.matmul(pav[:], lhsT=v_sb[:, kt, h, :], rhs=pt[:],
                                 start=(kt == 0), stop=(kt == NKT - 1))
            # out^T [65, 512] -> transpose 128-col blocks, divide by denom row
            oT = opool.tile([65, 512], F32, tag="oT", name="oT")
            nc.scalar.copy(oT[:], pav[:])
            for qt in range(4):
                tp = ps_tile([128, 65])
                nc.tensor.transpose(tp[:], oT[:, qt * 128:(qt + 1) * 128],
                                    ident_sb[0:65, 0:65])
                rc = rcpool.tile([128, 1], F32, tag="rcp", name="rc")
                nc.vector.reciprocal(rc[:], tp[:, 64:65])
                nc.vector.tensor_scalar_mul(
                    ofin[:, qt, h * DH:(h + 1) * DH], tp[:, 0:DH], rc[:])

        nc.sync.dma_start(out=out.rearrange("(a p) c -> p a c", p=128),
                          in_=ofin[:])

    nc.finalize()
    return nc


_CACHE = {}


def _tables():
    """Host-precomputed rotary tables / masks / permutation constants."""
    inv_freq = 1.0 / (10000.0 ** (np.arange(0, DH, 2, dtype=np.float32) / DH))
    pos = np.arange(S, dtype=np.float32)
    ang = pos[None, :] * inv_freq[(np.arange(128) % DH) // 2, None]  # [128, S]
    sign = np.where(np.arange(128) % 2 == 0, -1.0, 1.0).astype(np.float32)
    cosk = np.cos(ang)
    sink = np.sin(ang) * sign[:, None]
    bf = ml_dtypes.bfloat16
    cosq = (cosk[:, :QL] * 0.125).astype(bf)
    sinq = (sink[:, :QL] * 0.125).astype(bf)

    jj = np.arange(128)[:, None, None]
    mm = np.arange(4)[None, :, None]
    ii = np.arange(QL)[None, None, :]
    maskm = (ii >= 128 * mm + jj + 1).astype(bf)  # [128, 4, 512]

    idx = np.arange(128)
    pswap = np.zeros((128, 128), np.float32)
    pswap[idx, idx ^ 1] = 1.0
    ident = np.eye(128, dtype=np.float32)
    return dict(cosk=cosk.astype(bf), sink=sink.astype(bf), cosq=cosq,
                sinq=sinq, maskm=maskm, pswap=pswap, ident=ident)


def kernel(x, Wq, Wk, Wv, q_len, _want_results=False):
    x = np.asarray(x)
    assert int(q_len) == QL and x.shape == (4, S, E)
    if "nc" not in _CACHE:
        _CACHE["nc"] = _build()
        _CACHE["tables"] = _tables()
    nc, tb = _CACHE["nc"], _CACHE["tables"]

    in_maps = []
    for core in range(8):
        b, g = core // 2, core % 2
        sl = slice(g * DALL, (g + 1) * DALL)
        in_maps.append(dict(
            xb=np.ascontiguousarray(x[b]),
            wk=np.ascontiguousarray(np.asarray(Wk)[sl]),
            wv=np.ascontiguousarray(np.asarray(Wv)[sl]),
            wq=np.ascontiguousarray(np.asarray(Wq)[sl]),
            **tb))

    res = run_bass_kernel_spmd(nc, in_maps, list(range(8)))

    out = np.empty((4, QL, E), np.float32)
    for core in range(8):
        b, g = core // 2, core % 2
        out[b, :, g * DALL:(g + 1) * DALL] = res.results[core]["out"]
    if _want_results:
        return out, res
    return out
